# revision 1
# baseline (speedup 1.0000x reference)
"""Canny edge detector on 8 Trainium2 NeuronCores — pure data-parallel (1 image/core).

Pipeline per core (image 1024x1024 f32):
  1. 5x5 Gaussian blur (separable: vertical then horizontal 5-tap, exact f32)
  2. Sobel gx, gy (separable 3-taps)
  3. NMS using squared magnitudes (no sqrt / atan2 needed: compares on msq
     and tan^2 thresholds are exactly equivalent)
  4. Hysteresis: 16 iterations of 3x3 binary dilation masked by weak, on
     bit-packed state (32 px/word) with per-row gutter words.

Layout: "multirow" — partition p holds image rows [8p+d] in its free
dimension, row pitch 1028 (2 zero gutter cols each side) so ALL 8-neighbor
shifts are free-dim AP offsets.  Vertical halos come from overlapping HBM
loads (img) and SBUF->SBUF DMA halo refreshes (blurred, msq, packed state).

SBUF is tight: tensors share tile-pool slots via tags (same tag = same
address, Tile serializes via dependencies).
"""
import numpy as np

import concourse.bass as bass
import concourse.mybir as mybir
from concourse.tile import TileContext
from concourse.bass_utils import run_bass_kernel_spmd

P = 128          # partitions
R = 8            # image rows per partition
H = W = 1024
RP = 1028        # row pitch (2 gutter cols + 1024 data + 2 gutter cols)
DOF = 2          # data column offset within a row slot

# packed layout: 32 px/word -> 32 data words + 1 zero gutter word per row
PW = 33
NDW = 32

# hysteresis packed tile: 1 margin + (J halo + 8 own + J halo) data rows + 1 margin
HJ = 2           # halo rows == refresh cadence (iterations between halo refreshes)
HNR = 2 + 8 + 2 * HJ
HD0 = 1          # first data row (halo-top) in packed tiles
HOWN = 1 + HJ    # first own row in packed tiles

F32 = mybir.dt.float32
U32 = mybir.dt.uint32
I32 = mybir.dt.int32
I8 = mybir.dt.int8

CSPLIT = 720     # data-column split between DVE (left) and GPSIMD (right)
WSPLIT = 22      # packed-word split between DVE and GPSIMD


def _f32_consts():
    ax = np.arange(5, dtype=np.float32) - np.float32(2.0)
    g = np.exp(-(ax ** 2) / np.float32(2.0)).astype(np.float32)
    g = (g / g.sum()).astype(np.float32)
    c1 = np.float32(np.tan(np.deg2rad(22.5)) ** 2)
    c2 = np.float32(np.tan(np.deg2rad(67.5)) ** 2)

    def sqrt_thresh(t):
        t = np.float32(t)
        x = np.float32(t) * np.float32(t)
        while np.sqrt(np.float32(x)) >= t:
            x = np.nextafter(x, np.float32(0.0), dtype=np.float32)
        while np.sqrt(np.float32(x)) < t:
            x = np.nextafter(x, np.float32(np.inf), dtype=np.float32)
        return np.float32(x)

    return g, c1, c2, sqrt_thresh(0.1), sqrt_thresh(0.2)


def build_canny(nc, tc, pool, img_d, out_d, stage=99):
    import os
    stage = int(os.environ.get("CANNY_STAGE", stage))
    from concourse.alu_op_type import AluOpType as A
    g, c1, c2, tlow, thigh = _f32_consts()
    ve = nc.vector
    gp = nc.gpsimd
    se = nc.scalar


    def bail():
        z = pool.tile([P, 8, W], F32, name="zz", tag="tzz")
        ve.memset(z[:, :, :], 0.0)
        nc.sync.dma_start(out=out_d.rearrange("(p r w) -> p r w", p=P, r=R),
                          in_=z[:, :, :])

    def halves():
        return ((ve, 0, CSPLIT), (gp, CSPLIT, W))

    def zero_gutters(eng, t, nr):
        eng.memset(t[:, 0:nr, 0:DOF], 0.0)
        eng.memset(t[:, 0:nr, DOF + W:RP], 0.0)

    # per-partition integer scalar constants for bitwise scalar_tensor_tensor
    # (python int immediates lower as f32 there, which the verifier rejects)
    cst = pool.tile([P, 4], U32, name="cst", tag="tcst")
    ve.memset(cst[:, 0:1], 1)
    ve.memset(cst[:, 1:2], 16)
    ve.memset(cst[:, 2:3], 31)
    C1A, C16A, C31A = cst[:, 0:1], cst[:, 1:2], cst[:, 2:3]

    # ---------------- constant plane: pow2 for packing ----------------
    pow2i = pool.tile([P, W], U32, name="pow2i", tag="tconst")
    gp.iota(pow2i[:, :], pattern=[[1, W]], base=0, channel_multiplier=0)
    ve.tensor_single_scalar(pow2i[:, :], pow2i[:, :], 15, op=A.bitwise_and)
    ve.tensor_single_scalar(pow2i[:, :], pow2i[:, :], 127, op=A.add)
    ve.tensor_single_scalar(pow2i[:, :], pow2i[:, :], 23, op=A.logical_shift_left)
    pow2f = pow2i.bitcast(F32)

    # ---------------- load image (rows 8p-2 .. 8p+10) ----------------
    img = pool.tile([P, 12, RP], F32, name="img", tag="A")
    # zero the halo rows everywhere first; the DMA loads below overwrite all
    # but the out-of-image rows of partitions 0 / 127 (compute ops cannot
    # start at partition 127, so do full-partition memsets before the loads)
    ve.memset(img[:, 0:2, :], 0.0)
    ve.memset(img[:, 10:12, :], 0.0)

    img_rows = img_d.rearrange("(n w) -> n w", w=W)
    img_win = bass.AP(img_d, (R - 2) * W, [[R * W, P - 2], [W, 12], [1, W]])
    nc.sync.dma_start(out=img[1:P - 1, :, DOF:DOF + W], in_=img_win)
    nc.sync.dma_start(out=img[0:1, 2:12, DOF:DOF + W],
                      in_=img_rows[0:10, :].rearrange("(p r) w -> p r w", p=1))
    nc.sync.dma_start(out=img[P - 1:P, 0:10, DOF:DOF + W],
                      in_=img_rows[H - 10:H, :].rearrange("(p r) w -> p r w", p=1))

    # ---------------- vertical 5-tap blur -> blurv (own 8 rows) ----------------
    blurv = pool.tile([P, 8, RP], F32, name="blurv", tag="B")
    zero_gutters(ve, blurv, 8)
    pa1 = pool.tile([P, 8, W], F32, name="pa1", tag="C")
    pa2 = pool.tile([P, 8, W], F32, name="pa2", tag="F")
    PSPL = 664  # 65/35 DVE/GP split for the pair adds
    for eng, c0, c1_ in ((ve, 0, PSPL), (gp, PSPL, W)):
        eng.tensor_tensor(pa1[:, :, c0:c1_], img[:, 1:9, DOF + c0:DOF + c1_],
                          img[:, 3:11, DOF + c0:DOF + c1_], op=A.add)
        eng.tensor_tensor(pa2[:, :, c0:c1_], img[:, 0:8, DOF + c0:DOF + c1_],
                          img[:, 4:12, DOF + c0:DOF + c1_], op=A.add)
    dst = blurv[:, :, DOF:DOF + W]
    ve.tensor_single_scalar(dst, img[:, 2:10, DOF:DOF + W], float(g[2]), op=A.mult)
    ve.scalar_tensor_tensor(dst, pa1[:, :, :], float(g[1]), dst,
                            op0=A.mult, op1=A.add)
    ve.scalar_tensor_tensor(dst, pa2[:, :, :], float(g[0]), dst,
                            op0=A.mult, op1=A.add)

    if stage <= 1:
        bail()
        return

    # ---------------- horizontal 5-tap blur -> blurred [10 rows, own at 1..9] ---
    blurred = pool.tile([P, 10, RP], F32, name="blurred", tag="A")
    pb1 = pool.tile([P, 8, W], F32, name="pb1", tag="C")
    pb2 = pool.tile([P, 8, W], F32, name="pb2", tag="F")
    for eng, c0, c1_ in ((ve, 0, PSPL), (gp, PSPL, W)):
        eng.tensor_tensor(pb1[:, :, c0:c1_],
                          blurv[:, :, DOF + c0 - 1:DOF + c1_ - 1],
                          blurv[:, :, DOF + c0 + 1:DOF + c1_ + 1], op=A.add)
        eng.tensor_tensor(pb2[:, :, c0:c1_],
                          blurv[:, :, DOF + c0 - 2:DOF + c1_ - 2],
                          blurv[:, :, DOF + c0 + 2:DOF + c1_ + 2], op=A.add)
    dst = blurred[:, 1:9, DOF:DOF + W]
    ve.tensor_single_scalar(dst, blurv[:, :, DOF:DOF + W], float(g[2]), op=A.mult)
    ve.scalar_tensor_tensor(dst, pb1[:, :, :], float(g[1]), dst,
                            op0=A.mult, op1=A.add)
    ve.scalar_tensor_tensor(dst, pb2[:, :, :], float(g[0]), dst,
                            op0=A.mult, op1=A.add)
    # halo refresh: row 0 <- p-1 own row 7 (tile row 8); row 9 <- p+1 own row 0 (tile row 1)
    ve.memset(blurred[:, 0:1, :], 0.0)
    ve.memset(blurred[:, 9:10, :], 0.0)
    nc.sync.dma_start(out=blurred[1:P, 0:1, DOF:DOF + W],
                      in_=blurred[0:P - 1, 8:9, DOF:DOF + W])
    nc.scalar.dma_start(out=blurred[0:P - 1, 9:10, DOF:DOF + W],
                        in_=blurred[1:P, 1:2, DOF:DOF + W])

    if stage <= 2:
        bail()
        return

    # ---------------- sobel vertical parts (own 8 rows) ----------------
    # wx = bl[r-1] + 2 bl[r] + bl[r+1] ; vy = bl[r+1] - bl[r-1]
    wx = pool.tile([P, 8, RP], F32, name="wx", tag="C")
    vy = pool.tile([P, 8, RP], F32, name="vy", tag="F")
    zero_gutters(ve, wx, 8)
    zero_gutters(gp, vy, 8)
    bl = lambda dr: blurred[:, dr:dr + 8, DOF:DOF + W]
    wx_d = wx[:, :, DOF:DOF + W]
    vy_d = vy[:, :, DOF:DOF + W]
    for eng, c0, c1_ in halves():
        eng.tensor_tensor(wx[:, :, DOF + c0:DOF + c1_],
                          blurred[:, 0:8, DOF + c0:DOF + c1_],
                          blurred[:, 2:10, DOF + c0:DOF + c1_], op=A.add)
    ve.scalar_tensor_tensor(wx_d, bl(1), 2.0, wx_d, op0=A.mult, op1=A.add)
    gp.tensor_tensor(vy_d, bl(2), bl(0), op=A.subtract)

    # ---------------- sobel horizontal parts ----------------
    gx = pool.tile([P, 8, RP], F32, name="gx", tag="B")
    gy = pool.tile([P, 8, RP], F32, name="gy", tag="A")
    gx_d = gx[:, :, DOF:DOF + W]
    gy_d = gy[:, :, DOF:DOF + W]
    for eng, c0, c1_ in halves():
        eng.tensor_tensor(gx[:, :, DOF + c0:DOF + c1_],
                          wx[:, :, DOF + c0 + 1:DOF + c1_ + 1],
                          wx[:, :, DOF + c0 - 1:DOF + c1_ - 1], op=A.subtract)
    gp.tensor_tensor(gy_d, vy[:, :, DOF - 1:DOF - 1 + W],
                     vy[:, :, DOF + 1:DOF + 1 + W], op=A.add)
    ve.scalar_tensor_tensor(gy_d, vy_d, 2.0, gy_d, op0=A.mult, op1=A.add)

    if stage <= 3:
        bail()
        return

    # ---------------- sign of gx*gy, squares, msq ----------------
    sm = pool.tile([P, 8, W], U32, name="sm", tag="C")
    ve.tensor_tensor(sm[:, :, :], gx.bitcast(U32)[:, :, DOF:DOF + W],
                     gy.bitcast(U32)[:, :, DOF:DOF + W], op=A.bitwise_xor)
    ve.tensor_single_scalar(sm[:, :, :], sm[:, :, :], 31,
                            op=A.logical_shift_right)

    se.square(gx_d, gx_d)   # sqx
    se.square(gy_d, gy_d)   # sqy
    sqx, sqy = gx, gy
    sqx_d, sqy_d = gx_d, gy_d

    # direction classes (int8 0/1): nb0 = sqy < c1*sqx ; nb2 = sqy >= c2*sqx
    nb0 = pool.tile([P, 8, W], I8, name="nb0", tag="G")
    nb2 = pool.tile([P, 8, W], I8, name="nb2", tag="Hh")
    ve.scalar_tensor_tensor(nb0[:, :, :], sqx_d, float(c1), sqy_d,
                            op0=A.mult, op1=A.is_gt)
    ve.scalar_tensor_tensor(nb2[:, :, :], sqx_d, float(c2), sqy_d,
                            op0=A.mult, op1=A.is_le)

    # msq [10 rows, own at 1..9] with DMA halo refresh
    msq = pool.tile([P, 10, RP], F32, name="msq", tag="F")
    zero_gutters(ve, msq, 10)
    for eng, c0, c1_ in halves():
        n = c1_ - c0
        eng.tensor_tensor(msq[:, 1:9, DOF + c0:DOF + c0 + n],
                          sqx[:, :, DOF + c0:DOF + c0 + n],
                          sqy[:, :, DOF + c0:DOF + c0 + n], op=A.add)
    ve.memset(msq[:, 0:1, :], 0.0)
    ve.memset(msq[:, 9:10, :], 0.0)
    nc.sync.dma_start(out=msq[1:P, 0:1, :], in_=msq[0:P - 1, 8:9, :])
    nc.scalar.dma_start(out=msq[0:P - 1, 9:10, :], in_=msq[1:P, 1:2, :])

    if stage <= 4:
        bail()
        return

    # ---------------- NMS: directional pair maxes + predicated select ----------
    def msq_sh(dr, dj):
        return msq[:, 1 + dr:9 + dr, DOF + dj:DOF + dj + W]

    M = pool.tile([P, 8, W], F32, name="M", tag="B")        # after sqx dead
    m_d2 = pool.tile([P, 8, W], F32, name="m_d2", tag="A")  # after sqy dead
    ve.tensor_tensor(M[:, :, :], msq_sh(-1, 1), msq_sh(1, -1), op=A.max)   # NE/SW
    ve.tensor_tensor(m_d2[:, :, :], msq_sh(-1, -1), msq_sh(1, 1), op=A.max)  # NW/SE
    ve.copy_predicated(M[:, :, :], sm[:, :, :], m_d2[:, :, :])

    m_ns = pool.tile([P, 8, W], F32, name="m_ns", tag="C")
    ve.tensor_tensor(m_ns[:, :, :], msq_sh(-1, 0), msq_sh(1, 0), op=A.max)
    ve.copy_predicated(M[:, :, :], nb2[:, :, :], m_ns[:, :, :])

    m_ew = pool.tile([P, 8, W], F32, name="m_ew", tag="C")
    ve.tensor_tensor(m_ew[:, :, :], msq_sh(0, 1), msq_sh(0, -1), op=A.max)
    ve.copy_predicated(M[:, :, :], nb0[:, :, :], m_ew[:, :, :])

    # keep = (M <= msq), in place over M
    ve.scalar_tensor_tensor(M[:, :, :], M[:, :, :], 1.0,
                            msq[:, 1:9, DOF:DOF + W], op0=A.mult, op1=A.is_le)
    keep = M
    v = pool.tile([P, 8, W], F32, name="v", tag="A")
    for eng, c0, c1_ in halves():
        eng.tensor_tensor(v[:, :, c0:c1_], msq[:, 1:9, DOF + c0:DOF + c1_],
                          keep[:, :, c0:c1_], op=A.mult)

    if stage <= 5:
        bail()
        return

    # ---------------- threshold + bit-pack weak / strong ----------------
    ps = pool.tile([P, HNR, PW], U32, name="ps", tag="tps")
    pw_ = pool.tile([P, HNR, PW], U32, name="pw_", tag="tpw")
    gp.memset(ps[:, :, :], 0)
    gp.memset(pw_[:, :, :], 0)

    wgt = pool.tile([P, 8, W], F32, name="wgt", tag="C")
    sgt = pool.tile([P, 8, W], F32, name="sgt", tag="F")
    p2 = pow2f.unsqueeze(1).broadcast_to([P, 8, W])
    ve.scalar_tensor_tensor(wgt[:, :, :], v[:, :, :], float(tlow),
                            p2, op0=A.is_ge, op1=A.mult)
    ve.scalar_tensor_tensor(sgt[:, :, :], v[:, :, :], float(thigh),
                            p2, op0=A.is_ge, op1=A.mult)

    hw_w = pool.tile([P, 8, 64], F32, name="hw_w", tag="G")
    hw_s = pool.tile([P, 8, 64], F32, name="hw_s", tag="Hh")
    ve.tensor_reduce(hw_w[:, :, :],
                     wgt.rearrange("p r (s k) -> p r s k", k=16),
                     axis=mybir.AxisListType.X, op=A.add)
    ve.tensor_reduce(hw_s[:, :, :],
                     sgt.rearrange("p r (s k) -> p r s k", k=16),
                     axis=mybir.AxisListType.X, op=A.add)
    hi_w = pool.tile([P, 8, 64], U32, name="hi_w", tag="th3")
    hi_s = pool.tile([P, 8, 64], U32, name="hi_s", tag="th4")
    ve.tensor_copy(hi_w[:, :, :], hw_w[:, :, :])
    ve.tensor_copy(hi_s[:, :, :], hw_s[:, :, :])

    hv_w = hi_w.rearrange("p r (s two) -> p r s two", two=2)
    hv_s = hi_s.rearrange("p r (s two) -> p r s two", two=2)
    ve.scalar_tensor_tensor(pw_[:, HOWN:HOWN + 8, 0:NDW], hv_w[:, :, :, 1], C16A,
                            hv_w[:, :, :, 0], op0=A.logical_shift_left,
                            op1=A.bitwise_or)
    ve.scalar_tensor_tensor(ps[:, HOWN:HOWN + 8, 0:NDW], hv_s[:, :, :, 1], C16A,
                            hv_s[:, :, :, 0], op0=A.logical_shift_left,
                            op1=A.bitwise_or)

    # ---------------- packed halos ----------------
    def refresh_halos(t):
        nc.sync.dma_start(out=t[1:P, HD0:HD0 + HJ, :],
                          in_=t[0:P - 1, HOWN + 8 - HJ:HOWN + 8, :])
        nc.scalar.dma_start(out=t[0:P - 1, HOWN + 8:HOWN + 8 + HJ, :],
                            in_=t[1:P, HOWN:HOWN + HJ, :])

    refresh_halos(pw_)
    refresh_halos(ps)

    if stage <= 6:
        bail()
        return

    # ---------------- 16 iterations of masked dilation (packed) --------------
    Vt = pool.tile([P, HNR, PW], U32, name="Vt", tag="tV")
    Ht = pool.tile([P, HNR, PW], U32, name="Ht", tag="tH")
    gp.memset(Vt[:, :, :], 0)
    gp.memset(Ht[:, :, :], 0)

    nd = 8 + 2 * HJ
    flat = {}

    def rows_sh(t, dr=0, dw=0):
        key = id(t)
        if key not in flat:
            flat[key] = t.rearrange("p r w -> p (r w)")
        base = (HD0 + dr) * PW + dw
        return flat[key][:, base:base + nd * PW].rearrange("p (r w) -> p r w", w=PW)

    def hyst_iter():
        V = Vt[:, HD0:HD0 + nd, :]
        Hh = Ht[:, HD0:HD0 + nd, :]
        ve.tensor_tensor(V, rows_sh(ps, -1), rows_sh(ps, 1), op=A.bitwise_or)
        ve.tensor_tensor(V, rows_sh(ps), V, op=A.bitwise_or)
        ve.scalar_tensor_tensor(Hh, V, C1A, V, op0=A.logical_shift_left,
                                 op1=A.bitwise_or)
        ve.scalar_tensor_tensor(Hh, V, C1A, Hh, op0=A.logical_shift_right,
                                 op1=A.bitwise_or)
        ve.scalar_tensor_tensor(Hh, rows_sh(Vt, 0, -1), C31A, Hh,
                                 op0=A.logical_shift_right, op1=A.bitwise_or)
        ve.scalar_tensor_tensor(Hh, rows_sh(Vt, 0, 1), C31A, Hh,
                                 op0=A.logical_shift_left, op1=A.bitwise_or)
        ve.tensor_tensor(ps[:, HD0:HD0 + nd, :], Hh,
                         pw_[:, HD0:HD0 + nd, :], op=A.bitwise_and)

    for it in range(16):
        hyst_iter()
        if (it + 1) % HJ == 0 and it < 15:
            refresh_halos(ps)

    if stage <= 7:
        bail()
        return

    # ---------------- unpack own rows -> f32 0/1 and store --------------------
    # bidx[j] = 31 - (j % 32): shift so target bit lands in the sign bit
    bidx = pool.tile([P, W], U32, name="bidx", tag="tconst")
    gp.iota(bidx[:, :], pattern=[[1, W]], base=0, channel_multiplier=0)
    ve.tensor_single_scalar(bidx[:, :], bidx[:, :], 31, op=A.bitwise_and)
    ve.tensor_single_scalar(bidx[:, :], bidx[:, :], 31, op=A.bitwise_xor)
    # (x & 31) ^ 31 == 31 - (x & 31) for 0 <= x&31 <= 31

    tub = pool.tile([P, 8, W], I32, name="tub", tag="C")
    own_words = ps[:, HOWN:HOWN + 8, 0:NDW]
    expanded = own_words.unsqueeze(3).broadcast_to([P, 8, NDW, 32])
    bidx_b = (bidx.bitcast(I32).rearrange("p (w k) -> p w k", k=32)
              .unsqueeze(1).broadcast_to([P, 8, NDW, 32]))
    ve.tensor_tensor(tub.rearrange("p r (w k) -> p r w k", k=32),
                     expanded.bitcast(I32), bidx_b, op=A.logical_shift_left)
    outf = pool.tile([P, 8, W], F32, name="outf", tag="B")
    ve.tensor_single_scalar(outf[:, :, :], tub[:, :, :], 0, op=A.is_lt)

    nc.sync.dma_start(out=out_d.rearrange("(p r w) -> p r w", p=P, r=R),
                      in_=outf[:, :, :])


_CACHE = {}


def _get_built():
    if "nc" not in _CACHE:
        from concourse import bacc
        nc = bacc.Bacc(None)
        img_d = nc.declare_dram_parameter("img", [H * W], F32, isOutput=False)
        out_d = nc.declare_dram_parameter("out", [H * W], F32, isOutput=True)
        with TileContext(nc) as tc:
            with tc.tile_pool(name="main", bufs=1) as pool:
                build_canny(nc, tc, pool, img_d, out_d)
        nc.finalize()
        _CACHE["nc"] = nc
    return _CACHE["nc"]


TRACE = False        # set True (e.g. from test.py) to capture an NTFF profile
LAST_RESULT = None   # BassKernelResults of the most recent run


def kernel(image):
    global LAST_RESULT
    image = np.ascontiguousarray(np.asarray(image), dtype=np.float32)
    B = image.shape[0]
    assert image.shape == (B, 1, H, W)
    nc = _get_built()
    in_maps = [{"img": image[i, 0].reshape(-1)} for i in range(B)]
    res = run_bass_kernel_spmd(nc, in_maps, core_ids=list(range(B)),
                               trace=TRACE)
    LAST_RESULT = res
    out = np.stack([r["out"].reshape(H, W) for r in res.results])
    return out[:, None].astype(np.float32)



# revision 25
# speedup vs baseline: 1.2726x; 1.2726x over previous
"""Canny edge detector on 8 Trainium2 NeuronCores — pure data-parallel (1 image/core).

Pipeline per core (image 1024x1024 f32):
  1. 5x5 Gaussian blur (separable: vertical then horizontal 5-tap, exact f32)
  2. Sobel gx, gy (separable 3-taps)
  3. NMS using squared magnitudes (no sqrt / atan2 needed)
  4. Hysteresis: 16 iterations of 3x3 binary dilation masked by weak, on
     bit-packed state (32 px/word) with per-row gutter words.

Layout: "multirow" — partition p holds image rows [8p+d] in its free
dimension, row pitch 1028 (2 zero gutter cols each side) so ALL 8-neighbor
shifts are free-dim AP offsets.  Vertical halos come from overlapping HBM
loads (img) and SBUF->SBUF DMA halo refreshes (blurred, msq, packed state).

Engine facts (BIR-verifier-probed): Pool/GPSIMD supports ONLY f32
tensor_tensor add/sub/mult, tensor_single_scalar mult/add/max/compares,
copy, memset, iota.  No STT, no TT max/compare, no bitwise/shifts.
DVE does everything; Act does single-input activations (copy-scale,
square, relu).  So:
  - TT add/sub/mult ops column-split DVE|Pool at 672 (rates 1.042 vs 1.984)
  - single-scalar ops split at 745 (0.521 vs 1.389)
  - fused STT combines (a*s + b) run on DVE for cols [0:782], decomposed
    TSS-mult + TT-add on Pool for the rest
  - NMS pair maxes: DVE TT-max [0:810], Pool sub + Act relu + Pool add
    beyond (max(a,b) = b + relu(a-b); <=1-ulp rounding, flips only exact
    NMS ties — probability ~1e-12 per pixel)
  - hysteresis bitwise loop is DVE-only (hardware restriction), with
    interior-first iterations after each halo exchange to hide DMA latency
"""
import numpy as np

import concourse.bass as bass
import concourse.mybir as mybir
from concourse.tile import TileContext
from concourse.bass_utils import run_bass_kernel_spmd

P = 128          # partitions
R = 8            # image rows per partition
H = W = 1024
RP = 1028        # row pitch (2 gutter cols + 1024 data + 2 gutter cols)
DOF = 2          # data column offset within a row slot

# packed layout: 32 px/word -> 32 data words + 1 zero gutter word per row
PW = 33
NDW = 32

# hysteresis packed tile: 1 margin + (J halo + 8 own + J halo) data rows + 1 margin
HJ = 2           # halo rows == refresh cadence (iterations between halo refreshes)
HNR = 2 + 8 + 2 * HJ
HD0 = 1          # first data row (halo-top) in packed tiles
HOWN = 1 + HJ    # first own row in packed tiles

F32 = mybir.dt.float32
U32 = mybir.dt.uint32
U16 = mybir.dt.uint16
I16 = mybir.dt.int16
I32 = mybir.dt.int32
I8 = mybir.dt.int8

# DVE | Pool column splits (DVE gets [0:r), Pool [r:W))
RA = 672     # TT add/sub/mult      (DVE 1.042 vs Pool 1.984 ns/elem)
RS = 745     # single-scalar ops    (DVE 0.521 vs Pool 1.389)
RC = 782     # STT combine vs Pool TSS-mult + TT-add  (1.042 vs 3.373)
RX = 810     # TT max vs Pool sub + Act relu + Pool add (1.042 vs 3.968)
RN = 840     # STT cmp vs Pool TSS-mult + TT-sub + TSS-cmp (1.042 vs 4.762)
RT = 390     # pack-tree level-1 split of 512 pair sums


def _f32_consts():
    ax = np.arange(5, dtype=np.float32) - np.float32(2.0)
    g = np.exp(-(ax ** 2) / np.float32(2.0)).astype(np.float32)
    g = (g / g.sum()).astype(np.float32)
    c1 = np.float32(np.tan(np.deg2rad(22.5)) ** 2)
    c2 = np.float32(np.tan(np.deg2rad(67.5)) ** 2)

    def sqrt_thresh(t):
        t = np.float32(t)
        x = np.float32(t) * np.float32(t)
        while np.sqrt(np.float32(x)) >= t:
            x = np.nextafter(x, np.float32(0.0), dtype=np.float32)
        while np.sqrt(np.float32(x)) < t:
            x = np.nextafter(x, np.float32(np.inf), dtype=np.float32)
        return np.float32(x)

    return g, c1, c2, sqrt_thresh(0.1), sqrt_thresh(0.2)


def build_canny(nc, tc, pool, img_d, out_d, stage=99):
    import os
    stage = int(os.environ.get("CANNY_STAGE", stage))
    from concourse.alu_op_type import AluOpType as A
    g, c1, c2, tlow, thigh = _f32_consts()
    ve = nc.vector
    gp = nc.gpsimd
    se = nc.scalar

    def bail():
        z = pool.tile([P, 8, W], F32, name="zz", tag="C")
        ve.memset(z[:, :, :], 0.0)
        nc.sync.dma_start(out=out_d.rearrange("(p r w) -> p r w", p=P, r=R),
                          in_=z[:, :, :])

    def sp(r):
        return ((ve, 0, r), (gp, r, W))

    def zero_gutters(eng, t, nr):
        eng.memset(t[:, 0:nr, 0:DOF], 0.0)
        eng.memset(t[:, 0:nr, DOF + W:RP], 0.0)

    def comb(mk_dst, mk_src, s):
        """dst += s*src: DVE fused STT on [0:RC]; Pool scales src in place
        (src must be dead afterwards) then adds, on [RC:W]."""
        ve.scalar_tensor_tensor(mk_dst(0, RC), mk_src(0, RC), float(s),
                                mk_dst(0, RC), op0=A.mult, op1=A.add)
        gp.tensor_single_scalar(mk_src(RC, W), mk_src(RC, W), float(s),
                                op=A.mult)
        gp.tensor_tensor(mk_dst(RC, W), mk_src(RC, W), mk_dst(RC, W),
                         op=A.add)

    # per-partition integer scalar constants for bitwise scalar_tensor_tensor
    cst = pool.tile([P, 4], U32, name="cst", tag="tcst")
    ve.memset(cst[:, 0:1], 1)
    ve.memset(cst[:, 1:2], 16)
    ve.memset(cst[:, 2:3], 31)
    C1A, C16A, C31A = cst[:, 0:1], cst[:, 1:2], cst[:, 2:3]

    # ---------------- load image (rows 8p-2 .. 8p+10) ----------------
    img = pool.tile([P, 12, W], F32, name="img", tag="A")
    # out-of-image halo rows must be zero; the loads below overwrite all but
    # partition 0 / 127 edges (compute ops cannot start at partition 127, so
    # full-partition memsets, one per engine, before the loads)
    ve.memset(img[:, 0:2, :], 0.0)
    gp.memset(img[:, 10:12, :], 0.0)  # full partitions: p127 can't be sliced

    img_rows = img_d.rearrange("(n w) -> n w", w=W)
    img_win = bass.AP(img_d, (R - 2) * W, [[R * W, P - 2], [W, 12], [1, W]])
    nc.sync.dma_start(out=img[1:P - 1, :, :], in_=img_win)
    nc.sync.dma_start(out=img[0:1, 2:12, :],
                      in_=img_rows[0:10, :].rearrange("(p r) w -> p r w", p=1))
    nc.sync.dma_start(out=img[P - 1:P, 0:10, :],
                      in_=img_rows[H - 10:H, :].rearrange("(p r) w -> p r w", p=1))

    # ---------------- vertical 5-tap blur -> blurv (own 8 rows) ----------------
    blurv = pool.tile([P, 8, RP], F32, name="blurv", tag="B")
    zero_gutters(ve, blurv, 8)
    pa1 = pool.tile([P, 8, W], F32, name="pa1", tag="C")
    pa2 = pool.tile([P, 8, W], F32, name="pa2", tag="F")
    for eng, c0, c1_ in sp(RA):
        eng.tensor_tensor(pa1[:, :, c0:c1_], img[:, 1:9, c0:c1_],
                          img[:, 3:11, c0:c1_], op=A.add)
        eng.tensor_tensor(pa2[:, :, c0:c1_], img[:, 0:8, c0:c1_],
                          img[:, 4:12, c0:c1_], op=A.add)
    se.mul(blurv[:, :, DOF:DOF + W], img[:, 2:10, :], float(g[2]))
    comb(lambda a, b: blurv[:, :, DOF + a:DOF + b],
         lambda a, b: pa1[:, :, a:b], g[1])
    comb(lambda a, b: blurv[:, :, DOF + a:DOF + b],
         lambda a, b: pa2[:, :, a:b], g[0])

    if stage <= 1:
        bail()
        return

    # ---------------- horizontal 5-tap blur -> blurred [10 rows, own at 1..9] ---
    blurred = pool.tile([P, 10, RP], F32, name="blurred", tag="A")
    pb1 = pool.tile([P, 8, W], F32, name="pb1", tag="C")
    pb2 = pool.tile([P, 8, W], F32, name="pb2", tag="F")
    for eng, c0, c1_ in sp(RA):
        eng.tensor_tensor(pb1[:, :, c0:c1_],
                          blurv[:, :, DOF + c0 - 1:DOF + c1_ - 1],
                          blurv[:, :, DOF + c0 + 1:DOF + c1_ + 1], op=A.add)
        eng.tensor_tensor(pb2[:, :, c0:c1_],
                          blurv[:, :, DOF + c0 - 2:DOF + c1_ - 2],
                          blurv[:, :, DOF + c0 + 2:DOF + c1_ + 2], op=A.add)
    se.mul(blurred[:, 1:9, DOF:DOF + W], blurv[:, :, DOF:DOF + W], float(g[2]))
    comb(lambda a, b: blurred[:, 1:9, DOF + a:DOF + b],
         lambda a, b: pb1[:, :, a:b], g[1])
    comb(lambda a, b: blurred[:, 1:9, DOF + a:DOF + b],
         lambda a, b: pb2[:, :, a:b], g[0])
    # halo refresh: row 0 <- p-1 own row 7 (tile row 8); row 9 <- p+1 own row 0
    ve.memset(blurred[:, 0:1, :], 0.0)
    ve.memset(blurred[:, 9:10, :], 0.0)
    nc.sync.dma_start(out=blurred[1:P, 0:1, DOF:DOF + W],
                      in_=blurred[0:P - 1, 8:9, DOF:DOF + W])
    nc.scalar.dma_start(out=blurred[0:P - 1, 9:10, DOF:DOF + W],
                        in_=blurred[1:P, 1:2, DOF:DOF + W])

    if stage <= 2:
        bail()
        return

    # ---------------- sobel vertical parts (own 8 rows) ----------------
    # wx = bl[r-1] + 2 bl[r] + bl[r+1] ; vy = bl[r+1] - bl[r-1]
    wx = pool.tile([P, 8, RP], F32, name="wx", tag="C")
    vy = pool.tile([P, 8, RP], F32, name="vy", tag="F")
    zero_gutters(ve, wx, 8)
    zero_gutters(gp, vy, 8)
    for eng, c0, c1_ in sp(RA):
        eng.tensor_tensor(wx[:, :, DOF + c0:DOF + c1_],
                          blurred[:, 0:8, DOF + c0:DOF + c1_],
                          blurred[:, 2:10, DOF + c0:DOF + c1_], op=A.add)
        eng.tensor_tensor(vy[:, :, DOF + c0:DOF + c1_],
                          blurred[:, 2:10, DOF + c0:DOF + c1_],
                          blurred[:, 0:8, DOF + c0:DOF + c1_], op=A.subtract)
    # wx += 2*bl(center); Pool side scales blurred rows 1:9 in place (dead after)
    comb(lambda a, b: wx[:, :, DOF + a:DOF + b],
         lambda a, b: blurred[:, 1:9, DOF + a:DOF + b], 2.0)

    # ---------------- sobel horizontal parts ----------------
    gx = pool.tile([P, 8, RP], F32, name="gx", tag="B")
    gy = pool.tile([P, 8, RP], F32, name="gy", tag="A")
    gx_d = gx[:, :, DOF:DOF + W]
    gy_d = gy[:, :, DOF:DOF + W]
    for eng, c0, c1_ in sp(RA):
        eng.tensor_tensor(gx[:, :, DOF + c0:DOF + c1_],
                          wx[:, :, DOF + c0 + 1:DOF + c1_ + 1],
                          wx[:, :, DOF + c0 - 1:DOF + c1_ - 1], op=A.subtract)
        eng.tensor_tensor(gy[:, :, DOF + c0:DOF + c1_],
                          vy[:, :, DOF + c0 - 1:DOF + c1_ - 1],
                          vy[:, :, DOF + c0 + 1:DOF + c1_ + 1], op=A.add)
    # gy += 2*vy; Pool side scales vy in place (dead after)
    comb(lambda a, b: gy[:, :, DOF + a:DOF + b],
         lambda a, b: vy[:, :, DOF + a:DOF + b], 2.0)

    if stage <= 3:
        bail()
        return

    # ---------------- sign of gx*gy, squares, msq ----------------
    # diagonal-class mask: sm = (gx*gy < 0).  Product underflow to +-0 only
    # happens when msq is far below the weak threshold, where the NMS
    # direction choice can't affect the output.
    smw = pool.tile([P, 8, W], F32, name="smw", tag="C")
    sm = pool.tile([P, 8, W], I8, name="sm", tag="G2")
    for eng, c0, c1_ in sp(RA):
        eng.tensor_tensor(smw[:, :, c0:c1_], gx[:, :, DOF + c0:DOF + c1_],
                          gy[:, :, DOF + c0:DOF + c1_], op=A.mult)
    for eng, c0, c1_ in sp(RS):
        eng.tensor_single_scalar(sm[:, :, c0:c1_], smw[:, :, c0:c1_], 0.0,
                                 op=A.is_lt)

    se.square(gx_d, gx_d)   # sqx
    se.square(gy_d, gy_d)   # sqy
    sqx, sqy = gx, gy
    sqx_d, sqy_d = gx_d, gy_d

    # msq [10 rows, own at 1..9] with DMA halo refresh (before nb0/nb2 so the
    # Pool decompositions may clobber sqx afterwards)
    msq = pool.tile([P, 10, RP], F32, name="msq", tag="F")
    zero_gutters(ve, msq, 10)
    for eng, c0, c1_ in sp(RA):
        eng.tensor_tensor(msq[:, 1:9, DOF + c0:DOF + c1_],
                          sqx[:, :, DOF + c0:DOF + c1_],
                          sqy[:, :, DOF + c0:DOF + c1_], op=A.add)
    ve.memset(msq[:, 0:1, :], 0.0)
    ve.memset(msq[:, 9:10, :], 0.0)
    nc.sync.dma_start(out=msq[1:P, 0:1, :], in_=msq[0:P - 1, 8:9, :])
    nc.scalar.dma_start(out=msq[0:P - 1, 9:10, :], in_=msq[1:P, 1:2, :])

    # direction classes (int8 0/1): nb0 = sqy < c1*sqx ; nb2 = sqy >= c2*sqx
    # DVE: fused STT on [0:RN].  Pool on [RN:W]: t = c*sqx; d = t - sqy;
    # mask = sign test (exact: f32 subtract has exact sign).
    nb0 = pool.tile([P, 8, W], I8, name="nb0", tag="G")
    nb2 = pool.tile([P, 8, W], I8, name="nb2", tag="Hh")
    sc = pool.tile([P, 8, W - RN], F32, name="sc", tag="SC",
                   padded_shape=[P, 8, W - RX])
    ve.scalar_tensor_tensor(nb0[:, :, 0:RN], sqx_d[:, :, 0:RN], float(c1),
                            sqy_d[:, :, 0:RN], op0=A.mult, op1=A.is_gt)
    ve.scalar_tensor_tensor(nb2[:, :, 0:RN], sqx_d[:, :, 0:RN], float(c2),
                            sqy_d[:, :, 0:RN], op0=A.mult, op1=A.is_le)
    gp.tensor_single_scalar(sc[:, :, :], sqx_d[:, :, RN:W], float(c1),
                            op=A.mult)
    gp.tensor_tensor(sc[:, :, :], sc[:, :, :], sqy_d[:, :, RN:W],
                     op=A.subtract)
    gp.tensor_single_scalar(nb0[:, :, RN:W], sc[:, :, :], 0.0, op=A.is_gt)
    # nb2 Pool side: clobber sqx in place (dead after this)
    gp.tensor_single_scalar(sqx_d[:, :, RN:W], sqx_d[:, :, RN:W], float(c2),
                            op=A.mult)
    gp.tensor_tensor(sqx_d[:, :, RN:W], sqx_d[:, :, RN:W],
                     sqy_d[:, :, RN:W], op=A.subtract)
    gp.tensor_single_scalar(nb2[:, :, RN:W], sqx_d[:, :, RN:W], 0.0,
                            op=A.is_le)

    if stage <= 4:
        bail()
        return

    # ---------------- NMS: directional pair maxes + predicated select ----------
    def msq_sh(dr, dj, c0=0, c1_=W):
        return msq[:, 1 + dr:9 + dr, DOF + dj + c0:DOF + dj + c1_]

    # Pair maxes: m_d2/m_ns/m_ew split — DVE TT-max on [0:RX], Pool sub +
    # Act relu + Pool add decomp (max(a,b) = b + relu(a-b)) beyond.  M is
    # DVE-only so pred1 isn't gated on the slower Pool chain; the later
    # maxes' DVE parts slot between the predicated copies, which is exactly
    # when their Pool chains finish.
    M = pool.tile([P, 8, W], F32, name="M", tag="B")
    m_d2 = pool.tile([P, 8, W], F32, name="m_d2", tag="A")
    m_ns = pool.tile([P, 8, W], F32, name="m_ns", tag="C")

    def pmax_pool(dst, a_dr, a_dj, b_dr, b_dj):
        scm = pool.tile([P, 8, W - RX], F32, name="scm", tag="SC")
        gp.tensor_tensor(scm[:, :, :], msq_sh(a_dr, a_dj, RX, W),
                         msq_sh(b_dr, b_dj, RX, W), op=A.subtract)
        se.activation(scm[:, :, :], scm[:, :, :],
                      mybir.ActivationFunctionType.Relu)
        gp.tensor_tensor(dst[:, :, RX:W], scm[:, :, :],
                         msq_sh(b_dr, b_dj, RX, W), op=A.add)

    def pmax_dve(dst, a_dr, a_dj, b_dr, b_dj, c0=0, c1_=RX):
        ve.tensor_tensor(dst[:, :, c0:c1_], msq_sh(a_dr, a_dj, c0, c1_),
                         msq_sh(b_dr, b_dj, c0, c1_), op=A.max)

    pmax_pool(m_d2, -1, -1, 1, 1)       # Pool chains launch first
    pmax_pool(m_ns, -1, 0, 1, 0)
    pmax_dve(m_d2, -1, -1, 1, 1)
    pmax_dve(M, -1, 1, 1, -1, 0, W)     # full width on DVE
    HALF = 512
    for c0, c1_ in ((0, HALF), (HALF, W)):
        ve.copy_predicated(M[:, :, c0:c1_], sm[:, :, c0:c1_],
                           m_d2[:, :, c0:c1_])
    m_ew = pool.tile([P, 8, W], F32, name="m_ew", tag="A")
    pmax_pool(m_ew, 0, 1, 0, -1)
    pmax_dve(m_ns, -1, 0, 1, 0)
    for c0, c1_ in ((0, HALF), (HALF, W)):
        ve.copy_predicated(M[:, :, c0:c1_], nb2[:, :, c0:c1_],
                           m_ns[:, :, c0:c1_])
    pmax_dve(m_ew, 0, 1, 0, -1)
    for c0, c1_ in ((0, HALF), (HALF, W)):
        ve.copy_predicated(M[:, :, c0:c1_], nb0[:, :, c0:c1_],
                           m_ew[:, :, c0:c1_])

    if stage <= 5:
        bail()
        return

    # ---------------- threshold + bit-pack weak / strong ----------------
    ps = pool.tile([P, HNR, PW], U32, name="ps", tag="tps")
    pw_ = pool.tile([P, HNR, PW], U32, name="pw_", tag="tpw")
    gp.memset(ps[:, :, :], 0)
    gp.memset(pw_[:, :, :], 0)

    # keep/v folded into the thresholds: the edge bit is
    #   keep AND msq >= t  <=>  msq >= max(M, t)   (exact, t > 0)
    # then pairwise doubling tree:
    # t_{k+1}[j] = t_k[2j] + 2^(2^k) * t_k[2j+1]  -> 16-bit halfword sums
    # (exact f32 integer arithmetic, same bit order as 2^k weighting)
    def pack_mask(dest, thresh):
        src = pool.tile([P, 8, W], F32, name="thr", tag="C")
        scp = pool.tile([P, 8, W - RN], F32, name="scp", tag="SC",
                        padded_shape=[P, 8, W - RX])
        ve.scalar_tensor_tensor(src[:, :, 0:RN], M[:, :, 0:RN], float(thresh),
                                msq_sh(0, 0, 0, RN), op0=A.max, op1=A.is_le)
        gp.tensor_single_scalar(scp[:, :, :], M[:, :, RN:W], float(thresh),
                                op=A.max)
        gp.tensor_tensor(scp[:, :, :], msq_sh(0, 0, RN, W), scp[:, :, :],
                         op=A.subtract)
        gp.tensor_single_scalar(src[:, :, RN:W], scp[:, :, :], 0.0,
                                op=A.is_ge)
        lvl_tags = ("TS1", "C", "TS1", "C")
        mults = (2.0, 4.0, 16.0, 256.0)
        cur = src
        n = W // 2
        for li in range(4):
            nxt = pool.tile([P, 8, n], F32, name=f"tl{li}", tag=lvl_tags[li])
            cur_r = cur.rearrange("p r (j two) -> p r j two", two=2)
            if li == 0:
                # level 1 splits DVE | Pool (Pool scales odds in place)
                ve.scalar_tensor_tensor(nxt[:, :, 0:RT],
                                        cur_r[:, :, 0:RT, 1], mults[0],
                                        cur_r[:, :, 0:RT, 0],
                                        op0=A.mult, op1=A.add)
                gp.tensor_single_scalar(cur_r[:, :, RT:n, 1],
                                        cur_r[:, :, RT:n, 1], mults[0],
                                        op=A.mult)
                gp.tensor_tensor(nxt[:, :, RT:n], cur_r[:, :, RT:n, 1],
                                 cur_r[:, :, RT:n, 0], op=A.add)
            else:
                ve.scalar_tensor_tensor(nxt[:, :, :], cur_r[:, :, :, 1],
                                        mults[li], cur_r[:, :, :, 0],
                                        op0=A.mult, op1=A.add)
            cur = nxt
            n //= 2
        hi = pool.tile([P, 8, 64], U32, name="hi", tag="SC")
        ve.tensor_copy(hi[:, :, :], cur[:, :, :])
        hv = hi.rearrange("p r (s two) -> p r s two", two=2)
        ve.scalar_tensor_tensor(dest, hv[:, :, :, 1], C16A, hv[:, :, :, 0],
                                op0=A.logical_shift_left, op1=A.bitwise_or)

    pack_mask(pw_[:, HOWN:HOWN + 8, 0:NDW], tlow)
    pack_mask(ps[:, HOWN:HOWN + 8, 0:NDW], thigh)

    # ---------------- packed halos ----------------
    def refresh_halos(t):
        nc.sync.dma_start(out=t[1:P, HD0:HD0 + HJ, :],
                          in_=t[0:P - 1, HOWN + 8 - HJ:HOWN + 8, :])
        nc.scalar.dma_start(out=t[0:P - 1, HOWN + 8:HOWN + 8 + HJ, :],
                            in_=t[1:P, HOWN:HOWN + HJ, :])

    refresh_halos(pw_)
    refresh_halos(ps)

    if stage <= 6:
        bail()
        return

    # ---------------- 16 iterations of masked dilation (packed) --------------
    # Bitwise u32 ops are DVE-only on TRN2, so the whole packed loop runs on
    # the vector engine.  V has a zero gutter slot on each side so every
    # word-shift read stays in-tile.  Post-refresh iterations process
    # interior rows first so the halo-exchange DMA latency hides.
    nd = 8 + 2 * HJ
    Vd = pool.tile([P, HNR, NDW + 2], U32, name="Vd", tag="tV")  # gutter,0..31,gutter
    Hd = pool.tile([P, HNR, NDW], U32, name="Hd", tag="tH")
    ve.memset(Vd[:, :, :], 0)
    ve.memset(Hd[:, :, :], 0)

    def rsel(t, g, dr, w0, w1):
        # row-group selector: 'all' rows 1..12, 'core' 4..9, and the two
        # contiguous rim bands (STT ops only accept 2D/3D APs)
        if g == "all":
            return t[:, HD0 + dr:HD0 + nd + dr, w0:w1]
        if g == "core":
            return t[:, 4 + dr:10 + dr, w0:w1]
        if g == "rim1":
            return t[:, 1 + dr:4 + dr, w0:w1]
        return t[:, 10 + dr:13 + dr, w0:w1]

    def hyst_iter(groups=("all",)):
        for g in groups:
            V = rsel(Vd, g, 0, 1, NDW + 1)
            Hh = rsel(Hd, g, 0, 0, NDW)
            ve.tensor_tensor(V, rsel(ps, g, -1, 0, NDW),
                             rsel(ps, g, 1, 0, NDW), op=A.bitwise_or)
            ve.tensor_tensor(V, rsel(ps, g, 0, 0, NDW), V, op=A.bitwise_or)
            ve.scalar_tensor_tensor(Hh, V, C1A, V, op0=A.logical_shift_left,
                                    op1=A.bitwise_or)
            ve.scalar_tensor_tensor(Hh, V, C1A, Hh, op0=A.logical_shift_right,
                                    op1=A.bitwise_or)
            ve.scalar_tensor_tensor(Hh, rsel(Vd, g, 0, 0, NDW), C31A, Hh,
                                    op0=A.logical_shift_right, op1=A.bitwise_or)
            ve.scalar_tensor_tensor(Hh, rsel(Vd, g, 0, 2, NDW + 2), C31A, Hh,
                                    op0=A.logical_shift_left, op1=A.bitwise_or)
        for g in groups:
            ve.tensor_tensor(rsel(ps, g, 0, 0, NDW), rsel(Hd, g, 0, 0, NDW),
                             rsel(pw_, g, 0, 0, NDW), op=A.bitwise_and)

    n_iters = int(os.environ.get("CANNY_HYST_ITERS", 16))
    no_refresh = int(os.environ.get("CANNY_NO_REFRESH", 0))  # timing expt only
    hide = int(os.environ.get("CANNY_HIDE_REFRESH", 1))
    for it in range(n_iters):
        post_refresh = hide and it > 0 and it % HJ == 0 and not no_refresh
        hyst_iter(("core", "rim1", "rim2") if post_refresh else ("all",))
        if (it + 1) % HJ == 0 and it < n_iters - 1 and not no_refresh:
            refresh_halos(ps)

    if stage <= 7:
        bail()
        return

    # ---------------- unpack own rows -> f32 0/1 and store --------------------
    # ps own words viewed as u16 halfwords h (= pixels 16h..16h+15).  For each
    # bit position k: tub16[k][h] = hw[h] << (15-k)  (single-src imm-shift TSS
    # on packed u16 = 4x DVE mode), then sign test in the transposed view.
    own_hw = ps[:, HOWN:HOWN + 8, 0:NDW].bitcast(U16)   # [P, 8, 64]
    tub = pool.tile([P, 8, 16, 64], U16, name="tub", tag="TS1")
    for k in range(16):
        ve.tensor_single_scalar(tub[:, :, k, :], own_hw[:, :, :], 15 - k,
                                op=A.logical_shift_left)
    tub_px = tub.bitcast(I16).rearrange("p r k h -> p r h k")
    outf = pool.tile([P, 8, W], F32, name="outf", tag="B")
    out_r = out_d.rearrange("(p r w) -> p r w", p=P, r=R)
    outf_r = outf.rearrange("p r (h k) -> p r h k", k=16)
    for r0, r1 in ((0, 4), (4, 8)):
        ve.tensor_single_scalar(outf_r[:, r0:r1, :, :],
                                tub_px[:, r0:r1, :, :], 0, op=A.is_lt)
        nc.sync.dma_start(out=out_r[:, r0:r1, :], in_=outf[:, r0:r1, :])


_CACHE = {}


def _get_built():
    if "nc" not in _CACHE:
        from concourse import bacc
        nc = bacc.Bacc(None)
        img_d = nc.declare_dram_parameter("img", [H * W], F32, isOutput=False)
        out_d = nc.declare_dram_parameter("out", [H * W], F32, isOutput=True)
        with TileContext(nc) as tc:
            with tc.tile_pool(name="main", bufs=1) as pool:
                build_canny(nc, tc, pool, img_d, out_d)
        nc.finalize()
        _CACHE["nc"] = nc
    return _CACHE["nc"]


TRACE = False        # set True (e.g. from test.py) to capture an NTFF profile
LAST_RESULT = None   # BassKernelResults of the most recent run


def kernel(image):
    global LAST_RESULT
    image = np.ascontiguousarray(np.asarray(image), dtype=np.float32)
    B = image.shape[0]
    assert image.shape == (B, 1, H, W)
    nc = _get_built()
    in_maps = [{"img": image[i, 0].reshape(-1)} for i in range(B)]
    res = run_bass_kernel_spmd(nc, in_maps, core_ids=list(range(B)),
                               trace=TRACE)
    LAST_RESULT = res
    out = np.stack([r["out"].reshape(H, W) for r in res.results])
    return out[:, None].astype(np.float32)
